# revision 19
# baseline (speedup 1.0000x reference)
"""Trainium2 Bass kernel for masked BasicBlock (grouped conv3x3 -> BN -> ReLU
-> masked grouped conv3x3 -> BN -> +residual -> ReLU).

Strategy: data-parallel over batch across 8 NeuronCores (2 images/core);
grouped conv mapped to accumulating matmuls over a zero-padded SBUF image
layout; global training-mode BN stats via two 2KB AllReduces (hardware
bn_stats/bn_aggr per core); bf16 matmul operands.

Conv mapping per 128-channel group-pair, per 8-row output tile (N=448):
  - input tiles C_g = [ci(64) ; ci(64) shifted +1 row] so one K=128 matmul
    covers two dy taps at once; the third dy tap runs as K=64 on C_g[0:64].
  - two groups' M=64 matmuls are issued back-to-back at col positions 0/64 so
    they execute concurrently on disjoint PE array columns (~2x).
  - 6 matmul slots per tile instead of 9.

Host-side prep (part of kernel()): weight repacking to lhsT layouts, mask
expansion, conv1 input pre-masking (x*m), bf16 casts, and building the
padded + row-shifted duplicated conv1 input layout. Conv2's masking depends
on conv1 output and runs on-device.

Self-contained: hardcodes shapes from the problem spec.
"""
from contextlib import ExitStack

import numpy as np
import ml_dtypes

import concourse.bacc as bacc
import concourse.bass as bass
import concourse.mybir as mybir
from concourse.tile import TileContext
from concourse.bass_utils import run_bass_kernel_spmd

F32 = mybir.dt.float32
BF16 = mybir.dt.bfloat16
AF = mybir.ActivationFunctionType
ALU = mybir.AluOpType

N_CORES = 8
IMG = 2              # images per core
CIN = 256
G = 4
PAIRS = 2            # pairs of channel groups (128 ch each)
H = W = 56
PH, PW = 59, 58      # padded rows / cols (rows 0,57,58 and cols 0,57 zero)
PADN = PH * PW       # 3422
INT0 = PW            # flat offset of padded row 1
INTN = 56 * PW       # 3248: rows 1..56, all 58 cols
ROWT = 7             # 8-row output tiles per image
TN = 8 * W           # 448 pixels per psum tile
EPS = 1e-5
N_CORE_CNT = IMG * H * W
N_TOT = 16 * H * W

_prog_cache = {}


def _sub_ap(base, off, dims):
    """Custom free-dim access pattern on an existing AP (keeps partition dim)."""
    return bass.AP(
        tensor=base.tensor,
        offset=base.offset + off,
        ap=[list(base.ap[0])] + [list(d) for d in dims],
    )


def _build_program():
    nc = bacc.Bacc(num_devices=N_CORES)

    # conv1 input: host-premasked, padded, duplicated-shifted layout per group
    xd_d = nc.dram_tensor("xmdup", [IMG, G, 128, PADN], BF16, kind="ExternalInput")
    xr_d = nc.dram_tensor("xres", [IMG, CIN, H, W], BF16, kind="ExternalInput")
    y_d = nc.dram_tensor("y", [IMG, CIN, H, W], F32, kind="ExternalOutput")
    # pair-tap weights: lhsT [k=ci x {dy-1,dy0}, m=co64] per (conv,pair,g2,dx)
    wp_d = nc.dram_tensor("wpair", [2, PAIRS, 2, 3, 128, 64], BF16, kind="ExternalInput")
    # dy=+1 tap weights: lhsT [k=ci64, m=co64]
    w2_d = nc.dram_tensor("wdy2", [2, PAIRS, 2, 3, 128, 64], BF16, kind="ExternalInput")
    wz_d = nc.dram_tensor("wpz", [2, PAIRS, 128, 128], BF16, kind="ExternalInput")
    wz2_d = nc.dram_tensor("wpz2", [2, PAIRS, 128, 128], BF16, kind="ExternalInput")
    mr_d = nc.dram_tensor("mrow", [IMG, PAIRS, 128, 7 * PW], BF16, kind="ExternalInput")
    gb_d = nc.dram_tensor("gb", [2, PAIRS, 2, 128], F32, kind="ExternalInput")

    with TileContext(nc) as tc, ExitStack() as es:
        consts = es.enter_context(tc.tile_pool(name="consts", bufs=1))
        small = es.enter_context(tc.tile_pool(name="small", bufs=24))
        cp = es.enter_context(tc.tile_pool(name="cp", bufs=10))
        yp = es.enter_context(tc.tile_pool(name="yp", bufs=3))
        psp = es.enter_context(tc.tile_pool(name="psp", bufs=8, space="PSUM"))
        fop = es.enter_context(tc.tile_pool(name="fop", bufs=4))
        xrp = es.enter_context(tc.tile_pool(name="xrp", bufs=8))
        drp = es.enter_context(tc.tile_pool(name="drp", bufs=1, space="DRAM"))

        # ---- constants to SBUF ----
        wp_sb = {}
        w2_sb = {}
        for conv in range(2):
            for pair in range(PAIRS):
                for g2 in range(2):
                    for dx in range(3):
                        t = consts.tile([128, 64], BF16, tag=f"wp{conv}{pair}{g2}{dx}",
                                        name=f"wp{conv}{pair}{g2}{dx}")
                        nc.sync.dma_start(out=t[:], in_=wp_d[conv, pair, g2, dx])
                        wp_sb[(conv, pair, g2, dx)] = t
                        t2 = consts.tile([128, 64], BF16, tag=f"w2{conv}{pair}{g2}{dx}",
                                         name=f"w2{conv}{pair}{g2}{dx}")
                        nc.sync.dma_start(out=t2[:], in_=w2_d[conv, pair, g2, dx])
                        w2_sb[(conv, pair, g2, dx)] = t2

        wz_sb = {}
        for conv in range(2):
            for pair in range(PAIRS):
                t = consts.tile([128, 128], BF16, tag=f"wz{conv}{pair}",
                                name=f"wz{conv}{pair}")
                nc.sync.dma_start(out=t[:], in_=wz_d[conv, pair])
                wz_sb[(conv, pair)] = t
                t2 = consts.tile([128, 128], BF16, tag=f"wz2{conv}{pair}",
                                 name=f"wz2{conv}{pair}")
                nc.sync.dma_start(out=t2[:], in_=wz2_d[conv, pair])
                wz_sb[(conv, pair, "stop")] = t2

        mr_sb = {}
        for img in range(IMG):
            for pair in range(PAIRS):
                t = consts.tile([128, 7 * PW], BF16, tag=f"mr{img}{pair}",
                                name=f"mr{img}{pair}")
                nc.sync.dma_start(out=t[:], in_=mr_d[img, pair])
                mr_sb[(img, pair)] = t

        gam_sb = {}
        bet_sb = {}
        for conv in range(2):
            for pair in range(PAIRS):
                tg = consts.tile([128, 1], F32, tag=f"gam{conv}{pair}",
                                 name=f"gam{conv}{pair}")
                nc.sync.dma_start(
                    out=tg[:], in_=gb_d[conv, pair, 0].rearrange("(p o) -> p o", o=1))
                tb = consts.tile([128, 1], F32, tag=f"bet{conv}{pair}",
                                 name=f"bet{conv}{pair}")
                nc.sync.dma_start(
                    out=tb[:], in_=gb_d[conv, pair, 1].rearrange("(p o) -> p o", o=1))
                gam_sb[(conv, pair)] = tg
                bet_sb[(conv, pair)] = tb

        eps_sb = consts.tile([128, 1], F32, tag="eps", name="eps")
        nc.vector.memset(eps_sb[:], EPS)

        craw = {}
        for pair in range(PAIRS):
            for img in range(IMG):
                t = consts.tile([128, H * W], BF16, tag=f"cr{pair}{img}",
                                name=f"cr{pair}{img}")
                craw[(pair, img)] = t

        stats_sb = {
            (c, p): consts.tile([128, IMG * ROWT * 6], F32, tag=f"st{c}{p}",
                                name=f"st{c}{p}")
            for c in range(2) for p in range(PAIRS)
        }
        a_sb = {}
        b_sb = {}
        for conv in range(2):
            for pair in range(PAIRS):
                a_sb[(conv, pair)] = consts.tile([128, 1], F32, tag=f"a{conv}{pair}",
                                                 name=f"a{conv}{pair}")
                b_sb[(conv, pair)] = consts.tile([128, 1], F32, tag=f"b{conv}{pair}",
                                                 name=f"b{conv}{pair}")

        cc_in = {c: drp.tile([128, 2 * PAIRS], F32, tag=f"ccin{c}", name=f"ccin{c}")
                 for c in range(4)}
        cc_out = {c: drp.tile([128, 2 * PAIRS], F32, addr_space="Shared",
                              tag=f"ccout{c}", name=f"ccout{c}") for c in range(4)}

        # warm up collectives firmware so the real AllReduces hit the floor
        warm = small.tile([128, 2 * PAIRS], F32, tag="warm", name="warm")
        nc.vector.memset(warm[:], 0.0)
        nc.sync.dma_start(out=cc_in[2][:], in_=warm[:])
        nc.sync.dma_start(out=cc_in[3][:], in_=warm[:])
        for c in (2, 3):
            nc.gpsimd.collective_compute(
                "AllReduce", ALU.add,
                replica_groups=[list(range(N_CORES))],
                ins=[cc_in[c][:]], outs=[cc_out[c][:]],
            )
        warm2 = small.tile([128, 2 * PAIRS], F32, tag="warm2", name="warm2")
        nc.sync.dma_start(out=warm2[:], in_=cc_out[3][:])

        # ---------------- one conv layer ----------------
        def conv_block(conv):
            for img in range(IMG):
                for pair in range(PAIRS):
                    # ---- input tiles C_g0, C_g1 (padded, dup-shifted) ----
                    Cs = []
                    if conv == 0:
                        for g2 in range(2):
                            C = cp.tile([128, PADN], BF16, tag="C", name="C")
                            nc.sync.dma_start(
                                out=C[:], in_=xd_d[img, 2 * pair + g2])
                            Cs.append(C)
                    else:
                        yt = yp.tile([128, PADN], BF16, tag="yt", name="yt")
                        nc.vector.memset(_sub_ap(yt[:], 0, [[PW, PH]]), 0)
                        nc.vector.memset(_sub_ap(yt[:], PW - 1, [[PW, PH]]), 0)
                        nc.scalar.activation(
                            out=_sub_ap(yt[:], PW + 1, [[PW, H], [1, W]]),
                            in_=craw[(pair, img)][:],
                            func=AF.Relu,
                            bias=b_sb[(0, pair)][:],
                            scale=a_sb[(0, pair)][:],
                        )
                        for g2 in range(2):
                            mask_ap = _sub_ap(
                                mr_sb[(img, pair)][64 * g2:64 * (g2 + 1)], 0,
                                [[PW, 7], [0, 8], [1, PW]])
                            C = cp.tile([128, PADN], BF16, tag="C", name="C")
                            nc.vector.memset(C[0:64, 0:PW], 0)
                            nc.vector.memset(C[0:64, 57 * PW:PADN], 0)
                            nc.vector.memset(C[64:128, 56 * PW:58 * PW], 0)
                            ysrc = yt[64 * g2:64 * (g2 + 1), :]
                            nc.vector.tensor_mul(
                                C[0:64, INT0:INT0 + INTN],
                                ysrc[:, INT0:INT0 + INTN], mask_ap)
                            nc.vector.tensor_mul(
                                C[64:128, 0:INTN],
                                ysrc[:, INT0:INT0 + INTN], mask_ap)
                            Cs.append(C)

                    # ---- matmuls: 6 slots x 2 concurrent col-group MMs ----
                    # One M=128 start matmul per psum bank (pair-tap dx0 for g0
                    # in cols 0:64, zeros in 64:128) opens the accumulation
                    # group for the whole bank; everything else accumulates.
                    psums = [psp.tile([128, TN], F32, tag="ps", name="ps")
                             for _ in range(ROWT)]
                    for t in range(ROWT):
                        rhs = _sub_ap(Cs[0][:], (8 * t) * PW + 0,
                                      [[PW, 8], [1, W]])
                        nc.tensor.matmul(
                            psums[t][:], wz_sb[(conv, pair)][:], rhs,
                            start=True, stop=False, tile_position=(0, 0))
                        rhs = _sub_ap(Cs[1][:], (8 * t) * PW + 0,
                                      [[PW, 8], [1, W]])
                        nc.tensor.matmul(
                            psums[t][64:128, :],
                            wp_sb[(conv, pair, 1, 0)][:], rhs,
                            start=False, stop=False, tile_position=(0, 64))
                    for dx in range(3):
                        for t in range(ROWT):
                            for g2 in range(2):
                                rhs = _sub_ap(Cs[g2][:], (8 * t + 2) * PW + dx,
                                              [[PW, 8], [1, W]])
                                nc.tensor.matmul(
                                    psums[t][64 * g2:64 * (g2 + 1), :],
                                    w2_sb[(conv, pair, g2, dx)][:], rhs,
                                    start=False, stop=False,
                                    tile_position=(0, 64 * g2))
                    for t in range(ROWT):
                        for g2 in range(2):
                            rhs = _sub_ap(Cs[g2][:], (8 * t) * PW + 1,
                                          [[PW, 8], [1, W]])
                            nc.tensor.matmul(
                                psums[t][64 * g2:64 * (g2 + 1), :],
                                wp_sb[(conv, pair, g2, 1)][:], rhs,
                                start=False, stop=False,
                                tile_position=(0, 64 * g2))
                    for t in range(ROWT):
                        rhs = _sub_ap(Cs[0][:], (8 * t) * PW + 2,
                                      [[PW, 8], [1, W]])
                        nc.tensor.matmul(
                            psums[t][0:64, :],
                            wp_sb[(conv, pair, 0, 2)][:], rhs,
                            start=False, stop=False, tile_position=(0, 0))
                        rhs = _sub_ap(Cs[1][:], (8 * t) * PW + 2,
                                      [[PW, 8], [1, W]])
                        nc.tensor.matmul(
                            psums[t][:], wz_sb[(conv, pair, "stop")][:], rhs,
                            start=False, stop=True, tile_position=(0, 0))

                    # ---- evacuate + per-tile stats ----
                    for t in range(ROWT):
                        seg = craw[(pair, img)][:, TN * t:TN * (t + 1)]
                        nc.scalar.activation(out=seg, in_=psums[t][:], func=AF.Copy)
                        st = stats_sb[(conv, pair)][
                            :, (img * ROWT + t) * 6:(img * ROWT + t + 1) * 6]
                        nc.vector.bn_stats(out=st, in_=seg)

            # ---- global BN stats: aggregate -> AllReduce -> a,b ----
            sq = small.tile([128, 2 * PAIRS], F32, tag=f"sq{conv}", name=f"sq{conv}")
            for pair in range(PAIRS):
                mv = small.tile([128, 2], F32, tag="mv", name="mv")
                nc.vector.bn_aggr(
                    out=mv[:],
                    in_=stats_sb[(conv, pair)][:].rearrange("p (n s) -> p n s", s=6))
                nc.vector.tensor_scalar_mul(
                    sq[:, 2 * pair:2 * pair + 1], mv[:, 0:1], float(N_CORE_CNT))
                msq = small.tile([128, 1], F32, tag="msq", name="msq")
                nc.vector.tensor_mul(msq[:], mv[:, 0:1], mv[:, 0:1])
                nc.vector.tensor_add(msq[:], msq[:], mv[:, 1:2])
                nc.vector.tensor_scalar_mul(
                    sq[:, 2 * pair + 1:2 * pair + 2], msq[:], float(N_CORE_CNT))
            nc.sync.dma_start(out=cc_in[conv][:], in_=sq[:])
            nc.gpsimd.collective_compute(
                "AllReduce", ALU.add,
                replica_groups=[list(range(N_CORES))],
                ins=[cc_in[conv][:]], outs=[cc_out[conv][:]],
            )
            sq2 = small.tile([128, 2 * PAIRS], F32, tag=f"sq2{conv}", name=f"sq2{conv}")
            nc.sync.dma_start(out=sq2[:], in_=cc_out[conv][:])
            for pair in range(PAIRS):
                mu = small.tile([128, 1], F32, tag="mu", name="mu")
                nc.vector.tensor_scalar_mul(mu[:], sq2[:, 2 * pair:2 * pair + 1],
                                            1.0 / N_TOT)
                ex2 = small.tile([128, 1], F32, tag="ex2", name="ex2")
                nc.vector.tensor_scalar_mul(ex2[:], sq2[:, 2 * pair + 1:2 * pair + 2],
                                            1.0 / N_TOT)
                msq2 = small.tile([128, 1], F32, tag="msq2", name="msq2")
                nc.vector.tensor_mul(msq2[:], mu[:], mu[:])
                nc.vector.tensor_sub(ex2[:], ex2[:], msq2[:])      # biased var
                sd = small.tile([128, 1], F32, tag="sd", name="sd")
                nc.scalar.activation(out=sd[:], in_=ex2[:], func=AF.Sqrt,
                                     bias=eps_sb[:])
                rstd = small.tile([128, 1], F32, tag="rstd", name="rstd")
                nc.vector.reciprocal(out=rstd[:], in_=sd[:])
                nc.vector.tensor_mul(a_sb[(conv, pair)][:],
                                     gam_sb[(conv, pair)][:], rstd[:])
                t3 = small.tile([128, 1], F32, tag="t3", name="t3")
                nc.vector.tensor_mul(t3[:], a_sb[(conv, pair)][:], mu[:])
                nc.vector.tensor_sub(b_sb[(conv, pair)][:],
                                     bet_sb[(conv, pair)][:], t3[:])

        conv_block(0)
        conv_block(1)

        # ---------------- final: relu(a2*c2 + b2 + x) -> y ----------------
        HNW = H * W // 2
        # residual loads have no dependencies: issue them all up front so they
        # prefetch during conv2 instead of serializing into the tail
        xr_tiles = {}
        for img in range(IMG):
            for pair in range(PAIRS):
                for half in range(2):
                    xr = xrp.tile([128, HNW], BF16, tag="xr", name="xr")
                    nc.sync.dma_start(
                        out=xr[:],
                        in_=xr_d[img, 128 * pair:128 * (pair + 1),
                                 28 * half:28 * (half + 1)])
                    xr_tiles[(img, pair, half)] = xr
        for img in range(IMG):
            for pair in range(PAIRS):
                for half in range(2):
                    seg = slice(HNW * half, HNW * (half + 1))
                    o1 = fop.tile([128, HNW], F32, tag="o1", name="o1")
                    # u = a2*c2 + x  (one DVE op), then Relu(u + b2) on ACT
                    nc.vector.scalar_tensor_tensor(
                        out=o1[:],
                        in0=craw[(pair, img)][:, seg],
                        scalar=a_sb[(1, pair)][:],
                        in1=xr_tiles[(img, pair, half)][:],
                        op0=ALU.mult, op1=ALU.add)
                    nc.scalar.activation(out=o1[:], in_=o1[:], func=AF.Relu,
                                         bias=b_sb[(1, pair)][:])
                    nc.sync.dma_start(
                        out=y_d[img, 128 * pair:128 * (pair + 1),
                                28 * half:28 * (half + 1)],
                        in_=o1[:])

    nc.compile()
    return nc


def _pack_weights(w):
    """w [256,64,3,3] f32 -> (wpair [2,2,3,128,64], wdy2 [2,2,3,64,64]) bf16."""
    wpair = np.zeros([PAIRS, 2, 3, 128, 64], np.float32)
    wdy2 = np.zeros([PAIRS, 2, 3, 128, 64], np.float32)
    for pair in range(PAIRS):
        for g2 in range(2):
            g = 2 * pair + g2
            blk = w[64 * g:64 * (g + 1)]            # [64co, 64ci, 3, 3]
            for dx in range(3):
                wpair[pair, g2, dx, 0:64, :] = blk[:, :, 0, dx].T
                wpair[pair, g2, dx, 64:128, :] = blk[:, :, 1, dx].T
                wdy2[pair, g2, dx, 0:64, :] = blk[:, :, 2, dx].T
    bf = ml_dtypes.bfloat16
    return wpair.astype(bf), wdy2.astype(bf)


def _expand_mask_full(mask):
    """mask [N,4,7,7] -> [N,256,56,56] nearest-upsampled, channel-repeated."""
    m = np.repeat(np.repeat(mask, 8, axis=2), 8, axis=3)
    return np.repeat(m, CIN // G, axis=1)


def _pack_mask_rows(mask_core):
    """mask [IMG,4,7,7] -> mrow [IMG,PAIRS,128,7*58] bf16 (padded cols zero)."""
    mexp = np.repeat(mask_core, 8, axis=-1)         # [IMG,4,7,56]
    mrow = np.zeros([IMG, PAIRS, 128, 7, PW], np.float32)
    for pair in range(PAIRS):
        for g2 in range(2):
            g = 2 * pair + g2
            mrow[:, pair, 64 * g2:64 * (g2 + 1), :, 1:57] = mexp[:, g][:, None, :, :]
    return mrow.reshape(IMG, PAIRS, 128, 7 * PW).astype(ml_dtypes.bfloat16)


def _pack_xmdup(xm_core):
    """xm [IMG,256,56,56] (masked, f32) -> [IMG,G,128,PADN] bf16 padded dup."""
    xp = np.zeros([IMG, CIN, PH, PW], np.float32)
    xp[:, :, 1:57, 1:57] = xm_core
    out = np.zeros([IMG, G, 128, PH, PW], np.float32)
    for g in range(G):
        blk = xp[:, 64 * g:64 * (g + 1)]            # [IMG,64,PH,PW]
        out[:, g, 0:64] = blk
        out[:, g, 64:128, 0:PH - 1] = blk[:, :, 1:PH]   # shifted up one row
    return out.reshape(IMG, G, 128, PADN).astype(ml_dtypes.bfloat16)


def make_in_maps(x, mask, w1, gamma1, beta1, w2, gamma2, beta2):
    x = np.asarray(x, np.float32)
    mask = np.asarray(mask, np.float32)
    bf = ml_dtypes.bfloat16
    xm_full = x * _expand_mask_full(mask)
    xr_full = x.astype(bf)
    wp1, wd1 = _pack_weights(np.asarray(w1, np.float32))
    wp2, wd2 = _pack_weights(np.asarray(w2, np.float32))
    wpair = np.stack([wp1, wp2])
    wdy2 = np.stack([wd1, wd2])
    wpz = np.zeros([2, PAIRS, 128, 128], np.float32)
    wpz[:, :, :, 0:64] = wpair[:, :, 0, 0].astype(np.float32)
    wpz = wpz.astype(ml_dtypes.bfloat16)
    wpz2 = np.zeros([2, PAIRS, 128, 128], np.float32)
    wpz2[:, :, :, 64:128] = wpair[:, :, 1, 2].astype(np.float32)
    wpz2 = wpz2.astype(ml_dtypes.bfloat16)
    gb = np.zeros([2, PAIRS, 2, 128], np.float32)
    for pair in range(PAIRS):
        sl = slice(128 * pair, 128 * (pair + 1))
        gb[0, pair, 0] = np.asarray(gamma1, np.float32)[sl]
        gb[0, pair, 1] = np.asarray(beta1, np.float32)[sl]
        gb[1, pair, 0] = np.asarray(gamma2, np.float32)[sl]
        gb[1, pair, 1] = np.asarray(beta2, np.float32)[sl]

    in_maps = []
    for core in range(N_CORES):
        sl = slice(IMG * core, IMG * (core + 1))
        in_maps.append({
            "xmdup": _pack_xmdup(xm_full[sl]),
            "xres": np.ascontiguousarray(xr_full[sl]),
            "wpair": wpair,
            "wdy2": wdy2,
            "wpz": wpz,
            "wpz2": wpz2,
            "mrow": _pack_mask_rows(mask[sl]),
            "gb": gb,
        })
    return in_maps


def kernel(**inputs):
    if "nc" not in _prog_cache:
        _prog_cache["nc"] = _build_program()
    nc = _prog_cache["nc"]
    in_maps = make_in_maps(**inputs)
    res = run_bass_kernel_spmd(nc, in_maps, list(range(N_CORES)))
    y = np.concatenate([res.results[i]["y"] for i in range(N_CORES)], axis=0)
    return y.astype(np.float32)
